# revision 22
# baseline (speedup 1.0000x reference)
"""Row-normalize block-diagonal graph weights on 8 Trainium2 NeuronCores.

The reference computes, for edge_weight [K, N*N] and row [K*N*N] int32:
    deg      = segment_sum(w, row, num_segments=K*N)   # OOB rows dropped
    deg_inv  = where(deg > 0, 1/deg, 0)
    out      = deg_inv[row] * w                        # OOB rows clamped

In the expected inputs row[e] ~= e // N (block-diagonal graphs), but the
reference's own jnp.arange goes through float32 on CPU XLA, so a sparse
set of elements past 2^23 carries a neighboring (or out-of-range) row
id. The device kernel computes the dense per-block row sums + the
broadcast multiply (the memory-bound 256MB of traffic); the sparse
deviation set E = {e : row[e] != e//N} is folded in exactly via a tiny
host-computed degree-correction vector and a host-side fixup of the
~|E| affected output elements.

Sharding: pure data parallel over K (batch of graphs) — each core owns
K/8 = 4 graphs = a [4096, 1024] slab; no cross-core communication.
"""

import numpy as np

K = 32          # graphs in batch
N = 1024        # nodes per graph
NCORES = 8
KPC = K // NCORES          # graphs per core
ROWS = KPC * N             # 4096 source-node rows per core
NODES = K * N              # total segments
P = 128                    # SBUF partitions
Q = 8                      # consecutive rows per partition per slab
T = ROWS // (Q * P)        # 4 slabs of 4MB per core

_CACHE = {}


def _build_bass():
    """Build (once) the per-core Bass module:
    x[ROWS,N], corr[ROWS] -> y[ROWS,N] with y = x / (rowsum(x)+corr).

    Raw Bass (no Tile): this toolchain's walrus rejects any instruction
    carrying more than one semaphore wait (every ISA instruction struct
    has a single events slot), and Tile's sem assignment freely emits
    2+ on SBUF-slot or sem-lane reuse. With explicit raw-bass sems,
    every wait is its own instruction.

    Per-core plan: the whole 16MB shard resides in SBUF (T=4 slabs of
    4MB; partition p of slab t holds Q=8 consecutive DRAM rows, so all
    DMA access patterns are plain 2D with one contiguous 32KB run per
    partition). SP streams loads, DVE row-reduces + corrects + clamps +
    reciprocals + multiplies in place, PL streams stores.
    """
    if "nc" in _CACHE:
        return _CACHE["nc"]

    import concourse.bass as bass
    from concourse import mybir

    f32 = mybir.dt.float32
    nc = bass.Bass("TRN2", target_bir_lowering=False, debug=False,
                   num_devices=NCORES)
    x = nc.dram_tensor("x", [ROWS, N], f32, kind="ExternalInput").ap()
    corr = nc.dram_tensor("corr", [ROWS], f32, kind="ExternalInput").ap()
    y = nc.dram_tensor("y", [ROWS, N], f32, kind="ExternalOutput").ap()
    # slab t covers rows [t*P*Q, (t+1)*P*Q): partition p holds Q
    # consecutive DRAM rows -> one contiguous (Q*N*4)B run per partition
    xt = x.rearrange("(t p q) n -> t p (q n)", p=P, q=Q)
    yt = y.rearrange("(t p q) n -> t p (q n)", p=P, q=Q)
    ct = corr.rearrange("(t p q) -> t p q", p=P, q=Q)

    from contextlib import ExitStack
    with (
        nc.sbuf_tensor([P, T * Q * N], f32) as wall,
        nc.sbuf_tensor([P, T * Q], f32) as call_,
        nc.sbuf_tensor([P, T * Q], f32) as degall,
        nc.sbuf_tensor([P, T * Q], f32) as invall,
        nc.semaphore("s_cmp") as s_cmp,
        nc.semaphore("s_out") as s_out,
        nc.semaphore("s_corr") as s_corr,
        ExitStack() as _sems,
        nc.Block() as block,
    ):
        # one sem per load: completions of different DMAs on a queue
        # may land out of order, so cumulative thresholds on a shared
        # sem would be racy
        s_in = [_sems.enter_context(nc.semaphore(f"s_in{t}"))
                for t in range(T)]
        M = Q * N
        wap, cap = wall.ap(), call_.ap()
        degap, invap = degall.ap(), invall.ap()
        w = [wap[:, t * M:(t + 1) * M] for t in range(T)]
        cr = [cap[:, t * Q:(t + 1) * Q] for t in range(T)]
        deg = [degap[:, t * Q:(t + 1) * Q] for t in range(T)]
        inv = [invap[:, t * Q:(t + 1) * Q] for t in range(T)]

        @block.sync
        def _(sync):
            for t in range(T):
                sync.dma_start(out=cr[t], in_=ct[t]).then_inc(s_corr, 16)
            for t in range(T):
                sync.dma_start(out=w[t], in_=xt[t]).then_inc(s_in[t], 16)

        @block.vector
        def _(vector):
            vector.wait_ge(s_corr, 16 * T)
            for t in range(T):
                vector.wait_ge(s_in[t], 16)
                for q in range(Q):
                    vector.reduce_sum(out=deg[t][:, q:q + 1],
                                      in_=w[t][:, q * N:(q + 1) * N],
                                      axis=mybir.AxisListType.X)
                # DVE is a deep pipeline without interlocks: drain
                # between same-engine RAW-dependent ops
                vector.drain()
                vector.tensor_add(deg[t], deg[t], cr[t])
                vector.drain()
                # zero-degree rows: clamp so 1/deg stays finite; the
                # elements gathering a truly-zero degree are all zero
                # (nonneg weights) or host-fixed
                vector.tensor_scalar_max(deg[t], deg[t], 1e-30)
                vector.drain()
                vector.reciprocal(out=inv[t], in_=deg[t])
                vector.drain()
                for q in range(Q):
                    vector.tensor_scalar_mul(
                        w[t][:, q * N:(q + 1) * N],
                        w[t][:, q * N:(q + 1) * N],
                        inv[t][:, q:q + 1],
                    )
                # drain before signalling the store: the muls' sem
                # update can fire at retire, before their SBUF writes
                # are visible to the SDMA engines
                vector.drain().then_inc(s_cmp, 1)

        @block.gpsimd
        def _(gpsimd):
            for t in range(T):
                gpsimd.wait_ge(s_cmp, t + 1)
                gpsimd.dma_start(out=yt[t], in_=w[t]).then_inc(s_out, 16)
            gpsimd.wait_ge(s_out, 16 * T)

    _CACHE["nc"] = nc
    return nc


def _expected_row_pattern():
    if "base" not in _CACHE:
        _CACHE["base"] = (np.arange(K * N * N, dtype=np.int64) // N)
    return _CACHE["base"]


def _install_ntff_hook():
    """Recreate the NTFF profile hook the boot shim couldn't install
    (this image's antenv lacks axon_hooks). Safe no-op on failure."""
    import sys, types
    if "antenv.axon_hooks" in sys.modules:
        return
    try:
        from trn_agent_boot.trn_boot import _ntff_profile_via_ctypes
        hook = _ntff_profile_via_ctypes("/opt/axon/libaxon_pjrt.so")
        mod = types.ModuleType("antenv.axon_hooks")
        mod.get_axon_ntff_profile_hook = lambda: hook
        mod.set_axon_ntff_profile_hook = lambda h: None
        sys.modules["antenv.axon_hooks"] = mod
    except Exception:
        pass


def _run_spmd(edge_weight, corr, trace=False):
    from concourse.bass_utils import run_bass_kernel_spmd

    if trace:
        _install_ntff_hook()
    nc = _build_bass()
    ew = np.ascontiguousarray(np.asarray(edge_weight, dtype=np.float32))
    corr = np.ascontiguousarray(np.asarray(corr, dtype=np.float32))
    in_maps = [{"x": ew[c * KPC:(c + 1) * KPC].reshape(ROWS, N),
                "corr": corr[c * ROWS:(c + 1) * ROWS]}
               for c in range(NCORES)]
    res = run_bass_kernel_spmd(nc, in_maps, list(range(NCORES)), trace=trace)
    out = np.empty((K, N * N), dtype=np.float32)
    for c in range(NCORES):
        out[c * KPC:(c + 1) * KPC] = res.results[c]["y"].reshape(KPC, N * N)
    return out, res


def _prepare(edge_weight, row):
    """Host-side exact handling of E = {e : row[e] != e//N}.

    Returns (corr[NODES] f32 to add to the device row-sums,
             fixup_idx int64, fixup_val f32) so that
    rowsum+corr == segment_sum(w, row) and out[fixup_idx] = fixup_val
    reproduces deg_inv[clamped row] * w for the deviating elements.
    """
    w = edge_weight.reshape(-1)
    base = _expected_row_pattern()
    row = row.astype(np.int64, copy=False)
    E = np.flatnonzero(row != base)
    corr = np.zeros(NODES, dtype=np.float64)
    if E.size:
        wE = w[E].astype(np.float64)
        np.subtract.at(corr, base[E], wE)
        rE = row[E]
        valid = (rE >= 0) & (rE < NODES)
        np.add.at(corr, rE[valid], wE[valid])
    # accurate degrees for the fixup values
    deg = edge_weight.reshape(NODES, N).sum(axis=1, dtype=np.float64) + corr
    deg = deg.astype(np.float32)
    inv = np.where(deg > 0, np.float32(1.0) / deg, np.float32(0.0))
    if E.size:
        gather = np.clip(row[E], 0, NODES - 1)   # jnp OOB gather clamps
        fixup_val = (w[E] * inv[gather]).astype(np.float32)
    else:
        fixup_val = np.zeros(0, dtype=np.float32)
    return corr.astype(np.float32), E, fixup_val


def kernel(edge_weight, row, num_atom):
    edge_weight = np.asarray(edge_weight)
    row = np.asarray(row)
    if (edge_weight.shape != (K, N * N)
            or int(num_atom) != N
            or row.shape != (K * N * N,)):
        return _numpy_reference(edge_weight, row, int(num_atom))
    corr, E, fixup_val = _prepare(edge_weight, row)
    out, _ = _run_spmd(edge_weight, corr)
    if E.size:
        out.reshape(-1)[E] = fixup_val
    return out


def _numpy_reference(edge_weight, row, num_atom):
    """jnp-semantics fallback for unexpected shapes: scatter drops OOB,
    gather clamps."""
    Kb = edge_weight.shape[0]
    num_nodes = Kb * num_atom
    w = edge_weight.reshape(-1).astype(np.float32)
    row = row.astype(np.int64, copy=False)
    valid = (row >= 0) & (row < num_nodes)
    deg = np.zeros(num_nodes, dtype=np.float64)
    np.add.at(deg, row[valid], w[valid].astype(np.float64))
    deg = deg.astype(np.float32)
    deg_inv = np.where(deg > 0, np.float32(1.0) / deg, np.float32(0.0))
    out = deg_inv[np.clip(row, 0, num_nodes - 1)] * w
    return out.reshape(Kb, -1).astype(np.float32)


def bench(edge_weight, row, num_atom, trace=True):
    """Like kernel() but returns (output, BassKernelResults) with profiling."""
    edge_weight = np.asarray(edge_weight)
    row = np.asarray(row)
    corr, E, fixup_val = _prepare(edge_weight, row)
    out, res = _run_spmd(edge_weight, corr, trace=trace)
    if E.size:
        out.reshape(-1)[E] = fixup_val
    return out, res


# revision 25
# speedup vs baseline: 1.0212x; 1.0212x over previous
"""Row-normalize block-diagonal graph weights on 8 Trainium2 NeuronCores.

The reference computes, for edge_weight [K, N*N] and row [K*N*N] int32:
    deg      = segment_sum(w, row, num_segments=K*N)   # OOB rows dropped
    deg_inv  = where(deg > 0, 1/deg, 0)
    out      = deg_inv[row] * w                        # OOB rows clamped

In the expected inputs row[e] ~= e // N (block-diagonal graphs), but the
reference's own jnp.arange goes through float32 on CPU XLA, so a sparse
set of elements past 2^23 carries a neighboring (or out-of-range) row
id. The device kernel computes the dense per-block row sums + the
broadcast multiply (the memory-bound 256MB of traffic); the sparse
deviation set E = {e : row[e] != e//N} is folded in exactly via a tiny
host-computed degree-correction vector and a host-side fixup of the
~|E| affected output elements.

Sharding: pure data parallel over K (batch of graphs) — each core owns
K/8 = 4 graphs = a [4096, 1024] slab; no cross-core communication.
"""

import numpy as np

K = 32          # graphs in batch
N = 1024        # nodes per graph
NCORES = 8
KPC = K // NCORES          # graphs per core
ROWS = KPC * N             # 4096 source-node rows per core
NODES = K * N              # total segments
P = 128                    # SBUF partitions
Q = 4                      # consecutive rows per partition per slab
T = ROWS // (Q * P)        # 8 slabs of 2MB per core

_CACHE = {}


def _build_bass():
    """Build (once) the per-core Bass module:
    x[ROWS,N], corr[ROWS] -> y[ROWS,N] with y = x / (rowsum(x)+corr).

    Raw Bass (no Tile): this toolchain's walrus rejects any instruction
    carrying more than one semaphore wait (every ISA instruction struct
    has a single events slot), and Tile's sem assignment freely emits
    2+ on SBUF-slot or sem-lane reuse. With explicit raw-bass sems,
    every wait is its own instruction.

    Per-core plan: the whole 16MB shard resides in SBUF (T=4 slabs of
    4MB; partition p of slab t holds Q=8 consecutive DRAM rows, so all
    DMA access patterns are plain 2D with one contiguous 32KB run per
    partition). SP streams loads, DVE row-reduces + corrects + clamps +
    reciprocals + multiplies in place, PL streams stores.
    """
    if "nc" in _CACHE:
        return _CACHE["nc"]

    import concourse.bass as bass
    from concourse import mybir

    f32 = mybir.dt.float32
    nc = bass.Bass("TRN2", target_bir_lowering=False, debug=False,
                   num_devices=NCORES)
    x = nc.dram_tensor("x", [ROWS, N], f32, kind="ExternalInput").ap()
    corr = nc.dram_tensor("corr", [ROWS], f32, kind="ExternalInput").ap()
    y = nc.dram_tensor("y", [ROWS, N], f32, kind="ExternalOutput").ap()
    # slab t covers rows [t*P*Q, (t+1)*P*Q): partition p holds Q
    # consecutive DRAM rows -> one contiguous (Q*N*4)B run per partition
    xt = x.rearrange("(t p q) n -> t p (q n)", p=P, q=Q)
    yt = y.rearrange("(t p q) n -> t p (q n)", p=P, q=Q)
    ct = corr.rearrange("(t p q) -> p t q", p=P, q=Q)   # one 16KB DMA

    from contextlib import ExitStack
    with (
        nc.sbuf_tensor([P, T * Q * N], f32) as wall,
        nc.sbuf_tensor([P, T * Q], f32) as call_,
        nc.sbuf_tensor([P, T * Q], f32) as degall,
        nc.sbuf_tensor([P, T * Q], f32) as invall,
        nc.semaphore("s_cmp") as s_cmp,
        nc.semaphore("s_out") as s_out,
        nc.semaphore("s_corr") as s_corr,
        ExitStack() as _sems,
        nc.Block() as block,
    ):
        # one sem per load: completions of different DMAs on a queue
        # may land out of order, so cumulative thresholds on a shared
        # sem would be racy
        s_in = [_sems.enter_context(nc.semaphore(f"s_in{t}"))
                for t in range(T)]
        M = Q * N
        wap, cap = wall.ap(), call_.ap()
        degap, invap = degall.ap(), invall.ap()
        w = [wap[:, t * M:(t + 1) * M] for t in range(T)]
        cr = [cap[:, t * Q:(t + 1) * Q] for t in range(T)]
        deg = [degap[:, t * Q:(t + 1) * Q] for t in range(T)]
        inv = [invap[:, t * Q:(t + 1) * Q] for t in range(T)]

        cr3 = cap.rearrange("p (t q) -> p t q", q=Q)

        @block.sync
        def _(sync):
            sync.dma_start(out=cr3, in_=ct).then_inc(s_corr, 16)
            for t in range(T):
                sync.dma_start(out=w[t], in_=xt[t]).then_inc(s_in[t], 16)

        @block.vector
        def _(vector):
            vector.wait_ge(s_corr, 16)
            for t in range(T):
                vector.wait_ge(s_in[t], 16)
                for q in range(Q):
                    vector.reduce_sum(out=deg[t][:, q:q + 1],
                                      in_=w[t][:, q * N:(q + 1) * N],
                                      axis=mybir.AxisListType.X)
                # DVE is a deep pipeline without interlocks: drain
                # between same-engine RAW-dependent ops
                vector.drain()
                vector.tensor_add(deg[t], deg[t], cr[t])
                vector.drain()
                # zero-degree rows: clamp so 1/deg stays finite; the
                # elements gathering a truly-zero degree are all zero
                # (nonneg weights) or host-fixed
                vector.tensor_scalar_max(deg[t], deg[t], 1e-30)
                vector.drain()
                vector.reciprocal(out=inv[t], in_=deg[t])
                vector.drain()
                for q in range(Q):
                    vector.tensor_scalar_mul(
                        w[t][:, q * N:(q + 1) * N],
                        w[t][:, q * N:(q + 1) * N],
                        inv[t][:, q:q + 1],
                    )
                # drain before signalling the store: the muls' sem
                # update can fire at retire, before their SBUF writes
                # are visible to the SDMA engines
                vector.drain().then_inc(s_cmp, 1)

        @block.gpsimd
        def _(gpsimd):
            for t in range(T):
                gpsimd.wait_ge(s_cmp, t + 1)
                gpsimd.dma_start(out=yt[t], in_=w[t]).then_inc(s_out, 16)
            gpsimd.wait_ge(s_out, 16 * T)

    _CACHE["nc"] = nc
    return nc


def _expected_row_pattern():
    if "base" not in _CACHE:
        _CACHE["base"] = (np.arange(K * N * N, dtype=np.int64) // N)
    return _CACHE["base"]


def _install_ntff_hook():
    """Recreate the NTFF profile hook the boot shim couldn't install
    (this image's antenv lacks axon_hooks). Safe no-op on failure."""
    import sys, types
    if "antenv.axon_hooks" in sys.modules:
        return
    try:
        from trn_agent_boot.trn_boot import _ntff_profile_via_ctypes
        hook = _ntff_profile_via_ctypes("/opt/axon/libaxon_pjrt.so")
        mod = types.ModuleType("antenv.axon_hooks")
        mod.get_axon_ntff_profile_hook = lambda: hook
        mod.set_axon_ntff_profile_hook = lambda h: None
        sys.modules["antenv.axon_hooks"] = mod
    except Exception:
        pass


def _run_spmd(edge_weight, corr, trace=False):
    from concourse.bass_utils import run_bass_kernel_spmd

    if trace:
        _install_ntff_hook()
    nc = _build_bass()
    ew = np.ascontiguousarray(np.asarray(edge_weight, dtype=np.float32))
    corr = np.ascontiguousarray(np.asarray(corr, dtype=np.float32))
    in_maps = [{"x": ew[c * KPC:(c + 1) * KPC].reshape(ROWS, N),
                "corr": corr[c * ROWS:(c + 1) * ROWS]}
               for c in range(NCORES)]
    res = run_bass_kernel_spmd(nc, in_maps, list(range(NCORES)), trace=trace)
    out = np.empty((K, N * N), dtype=np.float32)
    for c in range(NCORES):
        out[c * KPC:(c + 1) * KPC] = res.results[c]["y"].reshape(KPC, N * N)
    return out, res


def _prepare(edge_weight, row):
    """Host-side exact handling of E = {e : row[e] != e//N}.

    Returns (corr[NODES] f32 to add to the device row-sums,
             fixup_idx int64, fixup_val f32) so that
    rowsum+corr == segment_sum(w, row) and out[fixup_idx] = fixup_val
    reproduces deg_inv[clamped row] * w for the deviating elements.
    """
    w = edge_weight.reshape(-1)
    base = _expected_row_pattern()
    row = row.astype(np.int64, copy=False)
    E = np.flatnonzero(row != base)
    corr = np.zeros(NODES, dtype=np.float64)
    if E.size:
        wE = w[E].astype(np.float64)
        np.subtract.at(corr, base[E], wE)
        rE = row[E]
        valid = (rE >= 0) & (rE < NODES)
        np.add.at(corr, rE[valid], wE[valid])
    # accurate degrees for the fixup values
    deg = edge_weight.reshape(NODES, N).sum(axis=1, dtype=np.float64) + corr
    deg = deg.astype(np.float32)
    inv = np.where(deg > 0, np.float32(1.0) / deg, np.float32(0.0))
    if E.size:
        gather = np.clip(row[E], 0, NODES - 1)   # jnp OOB gather clamps
        fixup_val = (w[E] * inv[gather]).astype(np.float32)
    else:
        fixup_val = np.zeros(0, dtype=np.float32)
    return corr.astype(np.float32), E, fixup_val


def kernel(edge_weight, row, num_atom):
    edge_weight = np.asarray(edge_weight)
    row = np.asarray(row)
    if (edge_weight.shape != (K, N * N)
            or int(num_atom) != N
            or row.shape != (K * N * N,)):
        return _numpy_reference(edge_weight, row, int(num_atom))
    corr, E, fixup_val = _prepare(edge_weight, row)
    out, _ = _run_spmd(edge_weight, corr)
    if E.size:
        out.reshape(-1)[E] = fixup_val
    return out


def _numpy_reference(edge_weight, row, num_atom):
    """jnp-semantics fallback for unexpected shapes: scatter drops OOB,
    gather clamps."""
    Kb = edge_weight.shape[0]
    num_nodes = Kb * num_atom
    w = edge_weight.reshape(-1).astype(np.float32)
    row = row.astype(np.int64, copy=False)
    valid = (row >= 0) & (row < num_nodes)
    deg = np.zeros(num_nodes, dtype=np.float64)
    np.add.at(deg, row[valid], w[valid].astype(np.float64))
    deg = deg.astype(np.float32)
    deg_inv = np.where(deg > 0, np.float32(1.0) / deg, np.float32(0.0))
    out = deg_inv[np.clip(row, 0, num_nodes - 1)] * w
    return out.reshape(Kb, -1).astype(np.float32)


def bench(edge_weight, row, num_atom, trace=True):
    """Like kernel() but returns (output, BassKernelResults) with profiling."""
    edge_weight = np.asarray(edge_weight)
    row = np.asarray(row)
    corr, E, fixup_val = _prepare(edge_weight, row)
    out, res = _run_spmd(edge_weight, corr, trace=trace)
    if E.size:
        out.reshape(-1)[E] = fixup_val
    return out, res


# revision 26
# speedup vs baseline: 1.0803x; 1.0578x over previous
"""Row-normalize block-diagonal graph weights on 8 Trainium2 NeuronCores.

The reference computes, for edge_weight [K, N*N] and row [K*N*N] int32:
    deg      = segment_sum(w, row, num_segments=K*N)   # OOB rows dropped
    deg_inv  = where(deg > 0, 1/deg, 0)
    out      = deg_inv[row] * w                        # OOB rows clamped

In the expected inputs row[e] ~= e // N (block-diagonal graphs), but the
reference's own jnp.arange goes through float32 on CPU XLA, so a sparse
set of elements past 2^23 carries a neighboring (or out-of-range) row
id. The device kernel computes the dense per-block row sums + the
broadcast multiply (the memory-bound 256MB of traffic); the sparse
deviation set E = {e : row[e] != e//N} is folded in exactly via a tiny
host-computed degree-correction vector and a host-side fixup of the
~|E| affected output elements.

Sharding: pure data parallel over K (batch of graphs) — each core owns
K/8 = 4 graphs = a [4096, 1024] slab; no cross-core communication.
"""

import numpy as np

K = 32          # graphs in batch
N = 1024        # nodes per graph
NCORES = 8
KPC = K // NCORES          # graphs per core
ROWS = KPC * N             # 4096 source-node rows per core
NODES = K * N              # total segments
P = 128                    # SBUF partitions
Q = 4                      # consecutive rows per partition per slab
T = ROWS // (Q * P)        # 8 slabs of 2MB per core

_CACHE = {}


def _build_bass():
    """Build (once) the per-core Bass module:
    x[ROWS,N], corr[ROWS] -> y[ROWS,N] with y = x / (rowsum(x)+corr).

    Raw Bass (no Tile): this toolchain's walrus rejects any instruction
    carrying more than one semaphore wait (every ISA instruction struct
    has a single events slot), and Tile's sem assignment freely emits
    2+ on SBUF-slot or sem-lane reuse. With explicit raw-bass sems,
    every wait is its own instruction.

    Per-core plan: the whole 16MB shard resides in SBUF (T=4 slabs of
    4MB; partition p of slab t holds Q=8 consecutive DRAM rows, so all
    DMA access patterns are plain 2D with one contiguous 32KB run per
    partition). SP streams loads, DVE row-reduces + corrects + clamps +
    reciprocals + multiplies in place, PL streams stores.
    """
    if "nc" in _CACHE:
        return _CACHE["nc"]

    import concourse.bass as bass
    from concourse import mybir

    f32 = mybir.dt.float32
    nc = bass.Bass("TRN2", target_bir_lowering=False, debug=False,
                   num_devices=NCORES)
    x = nc.dram_tensor("x", [ROWS, N], f32, kind="ExternalInput").ap()
    corr = nc.dram_tensor("corr", [P, T * Q], f32, kind="ExternalInput").ap()
    y = nc.dram_tensor("y", [ROWS, N], f32, kind="ExternalOutput").ap()
    # slab t covers rows [t*P*Q, (t+1)*P*Q): partition p holds Q
    # consecutive DRAM rows -> one contiguous (Q*N*4)B run per partition
    xt = x.rearrange("(t p q) n -> t p (q n)", p=P, q=Q)
    yt = y.rearrange("(t p q) n -> t p (q n)", p=P, q=Q)

    from contextlib import ExitStack
    with (
        nc.sbuf_tensor([P, T * Q * N], f32) as wall,
        nc.sbuf_tensor([P, T * Q], f32) as call_,
        nc.sbuf_tensor([P, T * Q], f32) as degall,
        nc.sbuf_tensor([P, T * Q], f32) as invall,
        nc.semaphore("s_cmp") as s_cmp,
        nc.semaphore("s_out") as s_out,
        nc.semaphore("s_corr") as s_corr,
        ExitStack() as _sems,
        nc.Block() as block,
    ):
        M = Q * N
        wap, cap = wall.ap(), call_.ap()
        degap, invap = degall.ap(), invall.ap()

        # chunks: (slab t, q0, qc). First/last slabs split into 1MB
        # halves so DVE warm-up and the final compute+store tail expose
        # less serial time.
        chunks = [(0, 0, 2), (0, 2, 2)] + [(t, 0, Q) for t in range(1, T - 1)] \
               + [(T - 1, 0, 2), (T - 1, 2, 2)]
        s_in = [_sems.enter_context(nc.semaphore(f"s_ld{i}"))
                for i in range(len(chunks))]

        def wslice(t, q0, qc):
            base = t * M + q0 * N
            return wap[:, base:base + qc * N]

        def sslice(ap_, t, q0, qc):
            base = t * Q + q0
            return ap_[:, base:base + qc]

        @block.sync
        def _(sync):
            for i, (t, q0, qc) in enumerate(chunks):
                sync.dma_start(out=wslice(t, q0, qc),
                               in_=xt[t][:, q0 * N:(q0 + qc) * N]
                               ).then_inc(s_in[i], 16)

        @block.vector
        def _(vector):
            vector.wait_ge(s_corr, 16)
            for i, (t, q0, qc) in enumerate(chunks):
                vector.wait_ge(s_in[i], 16)
                for q in range(q0, q0 + qc):
                    col = t * Q + q
                    vector.reduce_sum(out=degap[:, col:col + 1],
                                      in_=wap[:, col * N:(col + 1) * N],
                                      axis=mybir.AxisListType.X)
                # DVE is a deep pipeline without interlocks: drain
                # between same-engine RAW-dependent ops
                vector.drain()
                d = sslice(degap, t, q0, qc)
                vector.tensor_add(d, d, sslice(cap, t, q0, qc))
                vector.drain()
                # zero-degree rows: clamp so 1/deg stays finite
                vector.tensor_scalar_max(d, d, 1e-30)
                vector.drain()
                vector.reciprocal(out=sslice(invap, t, q0, qc), in_=d)
                vector.drain()
                for q in range(q0, q0 + qc):
                    col = t * Q + q
                    vector.tensor_scalar_mul(
                        wap[:, col * N:(col + 1) * N],
                        wap[:, col * N:(col + 1) * N],
                        invap[:, col:col + 1],
                    )
                # drain before signalling the store: the muls' sem
                # update can fire at retire, before their SBUF writes
                # are visible to the SDMA engines
                vector.drain().then_inc(s_cmp, 1)

        @block.gpsimd
        def _(gpsimd):
            # tiny contiguous-2D corr load on the (idle-at-start) PL
            # queue so it cannot clog the SP ring ahead of the big loads
            gpsimd.dma_start(out=cap[:, :], in_=corr).then_inc(s_corr, 16)
            for i, (t, q0, qc) in enumerate(chunks):
                gpsimd.wait_ge(s_cmp, i + 1)
                gpsimd.dma_start(out=yt[t][:, q0 * N:(q0 + qc) * N],
                                 in_=wslice(t, q0, qc)).then_inc(s_out, 16)
            gpsimd.wait_ge(s_out, 16 * len(chunks))

    _CACHE["nc"] = nc
    return nc


def _expected_row_pattern():
    if "base" not in _CACHE:
        _CACHE["base"] = (np.arange(K * N * N, dtype=np.int64) // N)
    return _CACHE["base"]


def _install_ntff_hook():
    """Recreate the NTFF profile hook the boot shim couldn't install
    (this image's antenv lacks axon_hooks). Safe no-op on failure."""
    import sys, types
    if "antenv.axon_hooks" in sys.modules:
        return
    try:
        from trn_agent_boot.trn_boot import _ntff_profile_via_ctypes
        hook = _ntff_profile_via_ctypes("/opt/axon/libaxon_pjrt.so")
        mod = types.ModuleType("antenv.axon_hooks")
        mod.get_axon_ntff_profile_hook = lambda: hook
        mod.set_axon_ntff_profile_hook = lambda h: None
        sys.modules["antenv.axon_hooks"] = mod
    except Exception:
        pass


def _run_spmd(edge_weight, corr, trace=False):
    from concourse.bass_utils import run_bass_kernel_spmd

    if trace:
        _install_ntff_hook()
    nc = _build_bass()
    ew = np.ascontiguousarray(np.asarray(edge_weight, dtype=np.float32))
    corr = np.ascontiguousarray(np.asarray(corr, dtype=np.float32))
    cperm = corr.reshape(NCORES, T, P, Q).transpose(0, 2, 1, 3) \
               .reshape(NCORES, P, T * Q)
    in_maps = [{"x": ew[c * KPC:(c + 1) * KPC].reshape(ROWS, N),
                "corr": np.ascontiguousarray(cperm[c])}
               for c in range(NCORES)]
    res = run_bass_kernel_spmd(nc, in_maps, list(range(NCORES)), trace=trace)
    out = np.empty((K, N * N), dtype=np.float32)
    for c in range(NCORES):
        out[c * KPC:(c + 1) * KPC] = res.results[c]["y"].reshape(KPC, N * N)
    return out, res


def _prepare(edge_weight, row):
    """Host-side exact handling of E = {e : row[e] != e//N}.

    Returns (corr[NODES] f32 to add to the device row-sums,
             fixup_idx int64, fixup_val f32) so that
    rowsum+corr == segment_sum(w, row) and out[fixup_idx] = fixup_val
    reproduces deg_inv[clamped row] * w for the deviating elements.
    """
    w = edge_weight.reshape(-1)
    base = _expected_row_pattern()
    row = row.astype(np.int64, copy=False)
    E = np.flatnonzero(row != base)
    corr = np.zeros(NODES, dtype=np.float64)
    if E.size:
        wE = w[E].astype(np.float64)
        np.subtract.at(corr, base[E], wE)
        rE = row[E]
        valid = (rE >= 0) & (rE < NODES)
        np.add.at(corr, rE[valid], wE[valid])
    # accurate degrees for the fixup values
    deg = edge_weight.reshape(NODES, N).sum(axis=1, dtype=np.float64) + corr
    deg = deg.astype(np.float32)
    inv = np.where(deg > 0, np.float32(1.0) / deg, np.float32(0.0))
    if E.size:
        gather = np.clip(row[E], 0, NODES - 1)   # jnp OOB gather clamps
        fixup_val = (w[E] * inv[gather]).astype(np.float32)
    else:
        fixup_val = np.zeros(0, dtype=np.float32)
    return corr.astype(np.float32), E, fixup_val


def kernel(edge_weight, row, num_atom):
    edge_weight = np.asarray(edge_weight)
    row = np.asarray(row)
    if (edge_weight.shape != (K, N * N)
            or int(num_atom) != N
            or row.shape != (K * N * N,)):
        return _numpy_reference(edge_weight, row, int(num_atom))
    corr, E, fixup_val = _prepare(edge_weight, row)
    out, _ = _run_spmd(edge_weight, corr)
    if E.size:
        out.reshape(-1)[E] = fixup_val
    return out


def _numpy_reference(edge_weight, row, num_atom):
    """jnp-semantics fallback for unexpected shapes: scatter drops OOB,
    gather clamps."""
    Kb = edge_weight.shape[0]
    num_nodes = Kb * num_atom
    w = edge_weight.reshape(-1).astype(np.float32)
    row = row.astype(np.int64, copy=False)
    valid = (row >= 0) & (row < num_nodes)
    deg = np.zeros(num_nodes, dtype=np.float64)
    np.add.at(deg, row[valid], w[valid].astype(np.float64))
    deg = deg.astype(np.float32)
    deg_inv = np.where(deg > 0, np.float32(1.0) / deg, np.float32(0.0))
    out = deg_inv[np.clip(row, 0, num_nodes - 1)] * w
    return out.reshape(Kb, -1).astype(np.float32)


def bench(edge_weight, row, num_atom, trace=True):
    """Like kernel() but returns (output, BassKernelResults) with profiling."""
    edge_weight = np.asarray(edge_weight)
    row = np.asarray(row)
    corr, E, fixup_val = _prepare(edge_weight, row)
    out, res = _run_spmd(edge_weight, corr, trace=trace)
    if E.size:
        out.reshape(-1)[E] = fixup_val
    return out, res
